# revision 27
# baseline (speedup 1.0000x reference)
"""Distributed TRN2 kernel for nn_Att_scores (attention score double-sum).

Math: the reference computes qkv = X @ W_qkv.T, splits q/k, and takes
scores = (q k^T * scale).sum(heads).sum(keys).  The head/key sums commute
with the matmuls, so exactly:
    Xsum[b]      = sum_n X[b, n, :]                      # [C]
    u[b]         = Wq^T (Wk Xsum[b])                     # [C]
    scores[b, n] = scale * X[b, n, :] . u[b]

Distribution (8 cores): X row-sharded (256 rows/batch per core); W split in
HALVES across SEngine pairs (core i holds rows h*384:(h+1)*384 of Wq and
Wk, h = i%2), so u = u_0 + u_1 with u_h = Wq_h^T (Wk_h Xsum).

Inputs are pre-cast to bf16 host-side in make_in_maps (the device matmuls
run bf16 anyway; halves HBM traffic to ~2MB/core).  Loads ride HWDGE
(nc.sync) as plain bf16 moves, X in 4 chunk-DMAs that pipeline with the
Xsum matmuls.

Layout discipline — every intermediate is computed directly in its
consumer's layout; there are no transposes between compute stages:
  * partial Xsum^T lands [c-part, (ck b)] via 24 matmuls with the X tiles
    stationary and a ones-vector streaming; one DVE copy feeds the DRAM
    bounce and the 6KB AllReduce runs in this transposed layout (the CC
    also serves as the global barrier for the pair exchange).
  * t_h[j, b] via lhsT = Wk_h^T 128-col slices (built by PE transposes in
    the AllReduce shadow, where engine time is free) x rhs = Xsum^T.
  * u_h^T[c', b] via lhsT = Wq_h 128-col slices x rhs = t_h, accumulated
    over the 3 j-tiles per ck chunk, written straight into the fp32 send
    buffer (remote_dma payloads must stay >= 48B/partition; bf16 payloads
    corrupt).
  * the pair exchange is one remote_dma_broadcast frame to the SEngine
    partner (XOR dtpb=1) on SWDGE queue 1 (own queue so the untriggered
    prep cannot stall the bulk loads), prepped in the load shadow and
    triggered when u^T is drained.
  * the final dot is a split 12-matmul PSUM accumulation: my u-half's 6
    matmuls run while the partner's half is in flight; the partner's 6 are
    gated by a bf16 cast carrying the attached remote-sem wait (waits are
    attached post-scheduling to the consumer's sync_info — standalone
    injected waits get merged by the legalizer and deadlock cross-core).
  * scale folds into the PSUM drains; X^T for the final dot is built by 24
    PE transposes in the AllReduce shadow.
End-of-NEFF quiesce waits drain the exchange semaphores so no descriptor
is in flight across executions.  Measured absmax relative error vs the
fp32 reference: 4.8e-3 (gate 2e-2).
"""

import numpy as np

B = 2
N = 2048
C = 768
H = 12
HD = C // H
SCALE = float(HD) ** -0.5
NCORES = 8
NS = N // NCORES          # 256 rows of each batch per core
CH = NS // 128            # 2 partition-chunks per batch per core
JT = C // 128             # 6 128-row tiles of full C
JH = JT // 2              # 3 tiles per W half

_compiled_nc = None


def _attach_wait(target_ins, sem, val):
    """Attach a HW-only semaphore wait directly to target_ins's dispatch
    conditions.  A standalone injected EventSemaphore gets merged with
    same-sem waits by the post-schedule legalizer (valid in its model where
    remote sems never advance, a cross-core deadlock for us); an extra
    SyncWait on the consumer itself is left alone.  Invisible to the Tile
    scheduling sim because it is added after scheduling."""
    import concourse.mybir as mb

    w = mb.SyncWait(
        sync_type="semaphore",
        id=sem.num,
        ant_name=sem.name,
        wait_mode="sem-ge-imm",
        wait_value=val,
    )
    si = target_ins.sync_info
    if si is None:
        target_ins.sync_info = mb.SyncInfo(on_wait=[w], on_update=[])
    else:
        si.on_wait.append(w)


def _build_and_compile(use_collective=True, repeats=1):
    import concourse.bass as bass  # noqa: F401
    import concourse.bacc as bacc
    import concourse.tile as tile
    import concourse.mybir as mybir
    from concourse import masks

    f32 = mybir.dt.float32
    bf16 = mybir.dt.bfloat16
    add = mybir.AluOpType.add
    nc = bacc.Bacc(
        "TRN2",
        target_bir_lowering=False,
        debug=False,
        num_devices=NCORES,
        num_swdge_queues=2,
    )

    x_d = nc.dram_tensor("x_in", [B, NS, C], bf16, kind="ExternalInput")
    # per-core W half: rows 0:384 = Wq_h, rows 384:768 = Wk_h (bf16, pre-cast)
    w_d = nc.dram_tensor("w_in", [C, C], bf16, kind="ExternalInput")
    out_d = nc.dram_tensor("scores_out", [B, NS], f32, kind="ExternalOutput")

    SLICES = ((0, 512), (512, 256))

    rsem = nc.alloc_semaphore("u_rsem")
    lsem = nc.alloc_semaphore("u_lsem")
    inject_specs = []  # (engine, sem, val, target_ins)

    with tile.TileContext(nc) as tc:
        with (
            tc.tile_pool(name="sbuf", bufs=1) as pool,
            tc.tile_pool(name="psum", bufs=1, space="PSUM") as psum,
            tc.tile_pool(name="dram", bufs=1, space="DRAM") as dram,
        ):
            x_bf = pool.tile([128, B * CH, C], bf16)   # [p, (b,ch), c]
            xT_sb = pool.tile([128, JT, B * NS], bf16)  # X^T: [c-part, ck, (b n)]
            wq_sb = pool.tile([128, JH, C], bf16)      # Wq half, row j=jt*128+p
            wk_sb = pool.tile([128, JH, C], bf16)      # Wk half natural
            wkT_sb = pool.tile([128, JT, 384], bf16)   # Wk_h^T: [c-part, ck, j]
            ones_red = pool.tile([128, 1], bf16)       # lhsT for row-sum
            ident_bf = pool.tile([128, 128], bf16)
            xspT_sb = pool.tile([128, JT * B], f32)    # partial Xsum^T
            xsumTf = pool.tile([128, JT * B], f32)     # reduced Xsum^T fp32
            ident_f = pool.tile([B, B], f32)
            xsumT_sb = pool.tile([128, JT, B], bf16)   # Xsum^T
            t2_sb = pool.tile([B, 384], bf16)          # t_h^T: [b, j-half]
            t_sb = pool.tile([128, JH, B], bf16)       # t_h: [j-part, jt, b]
            u2f_sb = pool.tile([B, C], f32)            # u_h fp32, b on parts
            uT_send = pool.tile([128, 2, JT * B], f32)  # parity-buffered send
            uT_recv = pool.tile([128, 2, JT * B], f32)  # partner's u half
            uTs_bf = pool.tile([128, JT * B], bf16)    # my u^T half, bf16
            uTr_bf = pool.tile([128, JT * B], bf16)    # partner's half, bf16
            out_row2 = pool.tile([B, B * NS], f32)

            for _rep in range(repeats):
                if _rep == 0:
                    nc.gpsimd.memset(ones_red[:], 1.0)
                    masks.make_identity(nc, ident_bf[:])
                    masks.make_identity(nc, ident_f[:])

                # ------- loads (HWDGE, plain bf16 — inputs pre-cast on host) -----
                for b in range(B):
                    for ch in range(CH):
                        nc.sync.dma_start(
                            x_bf[:, b * CH + ch, :],
                            x_d[b, ch * 128 : (ch + 1) * 128, :],
                        )
                nc.sync.dma_start(
                    wk_sb[:], w_d[384:768, :].rearrange("(t p) c -> p t c", p=128)
                )
                nc.sync.dma_start(
                    wq_sb[:], w_d[0:384, :].rearrange("(t p) c -> p t c", p=128)
                )

                # u-exchange prep: data-independent, descgen rides the load
                # shadow on Q7.  One frame to the SEngine partner (XOR 1).
                par = _rep % 2
                rd = [None] * NCORES
                rd[1] = (0, 1)
                # queue 1: untriggered preps must not stall the bulk loads
                # (queue 0) — SDMA drains each ring FIFO in order
                prep = nc.gpsimd.remote_dma_broadcast(
                    uT_recv[:, par, :], uT_send[:, par, :],
                    remote_sem=rsem, local_sem=lsem, rdests=rd, queue_num=1)

                # -------- partial Xsum^T (TensorE, lands [c-part, (ck b)]) --
                # lhsT = X tile (stationary): out[c, 0] = sum_n X[n, c];
                # the AllReduce then runs in transposed layout, so no
                # pre-bounce two-stage drain and no post-land transposes
                xs2_ps = psum.tile([128, JT * B], f32, tag="small", bufs=2)
                for b in range(B):
                    for ck in range(JT):
                        col = ck * B + b
                        for ch in range(CH):
                            nc.tensor.matmul(
                                xs2_ps[:, col : col + 1],
                                x_bf[:, b * CH + ch, ck * 128 : (ck + 1) * 128],
                                ones_red[:],
                                start=(ch == 0),
                                stop=(ch == CH - 1),
                            )
                nc.vector.tensor_copy(xspT_sb[:], xs2_ps[:])

                # ---------------- AllReduce of [B, C] partial Xsum ----------
                ar_in = dram.tile([128, JT * B], f32, name=f"ar_in{_rep}")
                ar_out = dram.tile(
                    [128, JT * B], f32, addr_space="Shared", name=f"ar_out{_rep}"
                )
                bounce = nc.scalar.dma_start(ar_in[:], xspT_sb[:])
                if use_collective:
                    nc.gpsimd.collective_compute(
                        "AllReduce",
                        add,
                        replica_groups=[list(range(NCORES))],
                        ins=[ar_in.opt()],
                        outs=[ar_out.opt()],
                    )
                else:
                    nc.scalar.dma_start(ar_out[:], ar_in[:])
                nc.scalar.dma_start(xsumTf[:], ar_out[:])
                nc.scalar.copy(
                    xsumT_sb[:].rearrange("p ck b -> p (ck b)"), xsumTf[:]
                )

                # ---------------- Wk_h^T via TensorE transpose --------------
                for ck in range(JT):
                    for jt in range(JH):
                        wt_ps = psum.tile(
                            [128, 128], bf16, tag="tr", bufs=2, name=f"wt{jt}_{ck}"
                        )
                        nc.tensor.transpose(
                            wt_ps[:],
                            wk_sb[:, jt, ck * 128 : (ck + 1) * 128],
                            ident_bf[:],
                        )
                        nc.vector.tensor_copy(
                            wkT_sb[:, ck, jt * 128 : (jt + 1) * 128], wt_ps[:]
                        )

                # HAM warm-up: keep the PE clock gate at full rate through the
                # transpose-heavy AllReduce window.
                warm_ps = psum.tile([1, 512], f32, tag="small", bufs=2)
                N_WARM = 12
                for i in range(N_WARM):
                    nc.tensor.matmul(
                        warm_ps[:],
                        ones_red[:],
                        wk_sb[:, i % JH, 0:512],
                        start=(i == 0),
                        stop=(i == N_WARM - 1),
                    )

                # ---------------- X^T via TensorE transpose ----------------
                for b in range(B):
                    for ch in range(CH):
                        col = (b * CH + ch) * 128
                        for ck in range(JT):
                            xt_ps2 = psum.tile(
                                [128, 128], bf16, tag="tr", bufs=2,
                                name=f"xtr{b}_{ch}_{ck}",
                            )
                            nc.tensor.transpose(
                                xt_ps2[:],
                                x_bf[:, b * CH + ch, ck * 128 : (ck + 1) * 128],
                                ident_bf[:],
                            )
                            if ck % 2 == 0:
                                nc.scalar.copy(
                                    xT_sb[:, ck, col : col + 128], xt_ps2[:]
                                )
                            else:
                                nc.vector.tensor_copy(
                                    xT_sb[:, ck, col : col + 128], xt_ps2[:]
                                )

                # ------- t_h[j, b] = sum_c Wk_h^T[c, j] Xsum^T[c, b] --------
                # lhsT = Wk_h^T 128-col slices (stationary), rhs = Xsum^T:
                # lands t directly with j on partitions — no drain/transpose
                for js in range(JH):
                    t_ps = psum.tile(
                        [128, 2], f32, tag="tr", bufs=2, name=f"td{js}"
                    )
                    for ck in range(JT):
                        nc.tensor.matmul(
                            t_ps[:],
                            wkT_sb[:, ck, js * 128 : (js + 1) * 128],
                            xsumT_sb[:, ck, :],
                            start=(ck == 0),
                            stop=(ck == JT - 1),
                        )
                    nc.vector.tensor_copy(t_sb[:, js, :], t_ps[:])

                # ------ u_h^T[c', b] = sum_j Wq_h[j, c'] t_h[j, b] ----------
                # lhsT = Wq_h 128-col slices (stationary), rhs = t_h: lands
                # u^T chunks straight into the send buffer — no drain and no
                # transposes between u and the exchange trigger
                tcopies = []
                for ck in range(JT):
                    u_ps = psum.tile(
                        [128, 2], f32, tag="tr", bufs=2, name=f"ud{ck}"
                    )
                    for jt in range(JH):
                        nc.tensor.matmul(
                            u_ps[:],
                            wq_sb[:, jt, ck * 128 : (ck + 1) * 128],
                            t_sb[:, jt, :],
                            start=(jt == 0),
                            stop=(jt == JH - 1),
                        )
                    cp = nc.vector.tensor_copy(
                        uT_send[:, par, ck * B : (ck + 1) * B], u_ps[:]
                    )
                    tcopies.append(cp)

                # fire the pair exchange once the send buffer is complete
                trig = nc.gpsimd.trigger_dma(count=1, queue_num=1)
                for cp in tcopies:
                    tile.add_dep_helper(trig.ins, cp.ins, sync=True,
                                        reason="u^T drained before trigger")

                # my half casts to bf16 immediately; the partner's cast
                # carries the attached remote-sem wait, so the first six
                # final matmuls (mine) overlap the partner's arrival
                cs_mine = nc.vector.tensor_copy(uTs_bf[:], uT_send[:, par, :])
                cs_part = nc.vector.tensor_copy(uTr_bf[:], uT_recv[:, par, :])
                tile.add_dep_helper(cs_part.ins, trig.ins, sync=False,
                                    reason="partner cast after trigger")
                # the attached rsem wait blocks the DVE stream; every DVE op
                # the AllReduce bounce needs must be scheduled before it,
                # else the cores deadlock through the collective
                tile.add_dep_helper(cs_part.ins, bounce.ins, sync=False,
                                    reason="rsem wait after the CC bounce")
                inject_specs.append(
                    ("vector", rsem, 2 * (_rep + 1), cs_part.ins))

                # ------- scores^T = scale * diag(u^T . X^T) -----------------
                sc_ps = psum.tile([B, 512], f32, tag="mid", bufs=1, name="sc")
                for half, ubuf in ((0, uTs_bf), (1, uTr_bf)):
                    for ck in range(JT):
                        nc.tensor.matmul(
                            sc_ps[:],
                            ubuf[:, ck * B : (ck + 1) * B],
                            xT_sb[:, ck, :],
                            start=(half == 0 and ck == 0),
                            stop=(half == 1 and ck == JT - 1),
                        )
                nc.scalar.mul(out_row2[:, 0:NS], sc_ps[:, 0:NS], SCALE)
                nc.scalar.dma_start(out_d[0:1, :], out_row2[0:1, 0:NS])
                nc.vector.tensor_scalar_mul(
                    out_row2[:, NS : 2 * NS], sc_ps[:, NS : 2 * NS], SCALE
                )
                nc.scalar.dma_start(out_d[1:2, :], out_row2[1:2, NS : 2 * NS])

    # HW-only cross-core waits, invisible to the scheduling sim
    for eng_name, sem, val, target in inject_specs:
        _attach_wait(target, sem, val)
    # end-of-NEFF quiesce: all sends flushed, all arrivals seen
    nc.gpsimd.wait_ge(lsem, 16 * repeats)
    nc.gpsimd.wait_ge(rsem, 2 * repeats)

    nc.compile()
    return nc


def _get_nc():
    global _compiled_nc
    if _compiled_nc is None:
        _compiled_nc = _build_and_compile()
    return _compiled_nc


def make_in_maps(X, W_qkv):
    import ml_dtypes

    X = np.asarray(X, dtype=np.float32).astype(ml_dtypes.bfloat16)
    W = np.asarray(W_qkv, dtype=np.float32).astype(ml_dtypes.bfloat16)
    assert X.shape == (B, N, C) and W.shape == (2 * C, C)
    halves = []
    for h in range(2):
        wq_h = W[h * 384 : (h + 1) * 384, :]
        wk_h = W[C + h * 384 : C + (h + 1) * 384, :]
        halves.append(np.ascontiguousarray(np.concatenate([wq_h, wk_h], axis=0)))
    return [
        {
            "x_in": np.ascontiguousarray(X[:, i * NS : (i + 1) * NS, :]),
            "w_in": halves[i % 2],
        }
        for i in range(NCORES)
    ]


def assemble_out(results):
    return np.concatenate(
        [results[i]["scores_out"] for i in range(NCORES)], axis=1
    ).astype(np.float32)


def kernel(X, W_qkv):
    from concourse import bass_utils

    nc = _get_nc()
    res = bass_utils.run_bass_kernel_spmd(
        nc, make_in_maps(X, W_qkv), core_ids=list(range(NCORES))
    )
    return assemble_out(res.results)


# revision 28
# speedup vs baseline: 1.2629x; 1.2629x over previous
"""Distributed TRN2 kernel for nn_Att_scores (attention score double-sum).

Math: the reference computes qkv = X @ W_qkv.T, splits q/k, and takes
scores = (q k^T * scale).sum(heads).sum(keys).  The head/key sums commute
with the matmuls, so exactly:
    Xsum[b]      = sum_n X[b, n, :]                      # [C]
    u[b]         = Wq^T (Wk Xsum[b])                     # [C]
    scores[b, n] = scale * X[b, n, :] . u[b]

Distribution (8 cores): X row-sharded (256 rows/batch per core); W split in
HALVES across SEngine pairs (core i holds rows h*384:(h+1)*384 of Wq and
Wk, h = i%2), so u = u_0 + u_1 with u_h = Wq_h^T (Wk_h Xsum).

Inputs are pre-cast to bf16 host-side in make_in_maps (the device matmuls
run bf16 anyway; halves HBM traffic to ~2MB/core).  Loads ride HWDGE
(nc.sync) as plain bf16 moves, X in 4 chunk-DMAs that pipeline with the
Xsum matmuls.

Layout discipline — every intermediate is computed directly in its
consumer's layout; there are no transposes between compute stages:
  * partial Xsum^T lands [c-part, (ck b)] via 24 matmuls with the X tiles
    stationary and a ones-vector streaming; one DVE copy feeds the DRAM
    bounce and the 6KB AllReduce runs in this transposed layout (the CC
    also serves as the global barrier for the pair exchange).
  * t_h[j, b] via lhsT = Wk_h^T 128-col slices (built by PE transposes in
    the AllReduce shadow, where engine time is free) x rhs = Xsum^T.
  * u_h^T[c', b] via lhsT = Wq_h 128-col slices x rhs = t_h, accumulated
    over the 3 j-tiles per ck chunk, written straight into the fp32 send
    buffer (remote_dma payloads must stay >= 48B/partition; bf16 payloads
    corrupt).
  * the pair exchange is one remote_dma_broadcast frame to the SEngine
    partner (XOR dtpb=1) on SWDGE queue 1 (own queue so the untriggered
    prep cannot stall the bulk loads), prepped in the load shadow and
    triggered when u^T is drained.
  * the final dot is a split 12-matmul PSUM accumulation: my u-half's 6
    matmuls run while the partner's half is in flight; the partner's 6 are
    gated by a bf16 cast carrying the attached remote-sem wait (waits are
    attached post-scheduling to the consumer's sync_info — standalone
    injected waits get merged by the legalizer and deadlock cross-core).
  * scale folds into the PSUM drains; X^T for the final dot is built by 24
    PE transposes in the AllReduce shadow.
End-of-NEFF quiesce waits drain the exchange semaphores so no descriptor
is in flight across executions.  Measured absmax relative error vs the
fp32 reference: 4.8e-3 (gate 2e-2).
"""

import numpy as np

B = 2
N = 2048
C = 768
H = 12
HD = C // H
SCALE = float(HD) ** -0.5
NCORES = 8
NS = N // NCORES          # 256 rows of each batch per core
CH = NS // 128            # 2 partition-chunks per batch per core
JT = C // 128             # 6 128-row tiles of full C
JH = JT // 2              # 3 tiles per W half

_compiled_nc = None


def _attach_wait(target_ins, sem, val):
    """Attach a HW-only semaphore wait directly to target_ins's dispatch
    conditions.  A standalone injected EventSemaphore gets merged with
    same-sem waits by the post-schedule legalizer (valid in its model where
    remote sems never advance, a cross-core deadlock for us); an extra
    SyncWait on the consumer itself is left alone.  Invisible to the Tile
    scheduling sim because it is added after scheduling."""
    import concourse.mybir as mb

    w = mb.SyncWait(
        sync_type="semaphore",
        id=sem.num,
        ant_name=sem.name,
        wait_mode="sem-ge-imm",
        wait_value=val,
    )
    si = target_ins.sync_info
    if si is None:
        target_ins.sync_info = mb.SyncInfo(on_wait=[w], on_update=[])
    else:
        si.on_wait.append(w)


def _build_and_compile(use_collective=True, repeats=1):
    import concourse.bass as bass  # noqa: F401
    import concourse.bacc as bacc
    import concourse.tile as tile
    import concourse.mybir as mybir
    from concourse import masks

    f32 = mybir.dt.float32
    bf16 = mybir.dt.bfloat16
    add = mybir.AluOpType.add
    nc = bacc.Bacc(
        "TRN2",
        target_bir_lowering=False,
        debug=False,
        num_devices=NCORES,
        num_swdge_queues=2,
    )

    x_d = nc.dram_tensor("x_in", [B, NS, C], bf16, kind="ExternalInput")
    # per-core W half: rows 0:384 = Wq_h, rows 384:768 = Wk_h (bf16, pre-cast)
    w_d = nc.dram_tensor("w_in", [C, C], bf16, kind="ExternalInput")
    out_d = nc.dram_tensor("scores_out", [B, NS], f32, kind="ExternalOutput")

    SLICES = ((0, 512), (512, 256))

    rsem = nc.alloc_semaphore("u_rsem")
    lsem = nc.alloc_semaphore("u_lsem")
    inject_specs = []  # (engine, sem, val, target_ins)

    with tile.TileContext(nc) as tc:
        with (
            tc.tile_pool(name="sbuf", bufs=1) as pool,
            tc.tile_pool(name="psum", bufs=1, space="PSUM") as psum,
            tc.tile_pool(name="dram", bufs=1, space="DRAM") as dram,
        ):
            x_bf = pool.tile([128, B * CH, C], bf16)   # [p, (b,ch), c]
            xT_sb = pool.tile([128, JT, B * NS], bf16)  # X^T: [c-part, ck, (b n)]
            wq_sb = pool.tile([128, JH, C], bf16)      # Wq half, row j=jt*128+p
            wk_sb = pool.tile([128, JH, C], bf16)      # Wk half natural
            wkT_sb = pool.tile([128, JT, 384], bf16)   # Wk_h^T: [c-part, ck, j]
            ones_red = pool.tile([128, 1], bf16)       # lhsT for row-sum
            ident_bf = pool.tile([128, 128], bf16)
            xspT_sb = pool.tile([128, JT * B], f32)    # partial Xsum^T
            xsumTf = pool.tile([128, JT * B], f32)     # reduced Xsum^T fp32
            ident_f = pool.tile([B, B], f32)
            xsumT_sb = pool.tile([128, JT, B], bf16)   # Xsum^T
            t2_sb = pool.tile([B, 384], bf16)          # t_h^T: [b, j-half]
            t_sb = pool.tile([128, JH, B], bf16)       # t_h: [j-part, jt, b]
            u2f_sb = pool.tile([B, C], f32)            # u_h fp32, b on parts
            uT_send = pool.tile([128, 2, JT * B], f32)  # parity-buffered send
            uT_recv = pool.tile([128, 2, JT * B], f32)  # partner's u half
            uTs_bf = pool.tile([128, JT * B], bf16)    # my u^T half, bf16
            uTr_bf = pool.tile([128, JT * B], bf16)    # partner's half, bf16
            out_row2 = pool.tile([B, B * NS], f32)

            for _rep in range(repeats):
                if _rep == 0:
                    nc.gpsimd.memset(ones_red[:], 1.0)
                    masks.make_identity(nc, ident_bf[:])
                    masks.make_identity(nc, ident_f[:])

                # ------- loads (HWDGE, plain bf16 — inputs pre-cast on host) -----
                for b in range(B):
                    for ch in range(CH):
                        nc.sync.dma_start(
                            x_bf[:, b * CH + ch, :],
                            x_d[b, ch * 128 : (ch + 1) * 128, :],
                        )
                nc.sync.dma_start(
                    wk_sb[:], w_d[384:768, :].rearrange("(t p) c -> p t c", p=128)
                )
                nc.sync.dma_start(
                    wq_sb[:], w_d[0:384, :].rearrange("(t p) c -> p t c", p=128)
                )

                # u-exchange prep: data-independent, descgen rides the load
                # shadow on Q7.  One frame to the SEngine partner (XOR 1).
                par = _rep % 2
                rd = [None] * NCORES
                rd[1] = (0, 1)
                # queue 1: untriggered preps must not stall the bulk loads
                # (queue 0) — SDMA drains each ring FIFO in order
                prep = nc.gpsimd.remote_dma_broadcast(
                    uT_recv[:, par, :], uT_send[:, par, :],
                    remote_sem=rsem, local_sem=lsem, rdests=rd, queue_num=1)

                # -------- partial Xsum^T (TensorE, lands [c-part, (ck b)]) --
                # lhsT = X tile (stationary): out[c, 0] = sum_n X[n, c];
                # the AllReduce then runs in transposed layout, so no
                # pre-bounce two-stage drain and no post-land transposes
                xs2_ps = psum.tile([128, JT * B], f32, tag="small", bufs=2)
                for b in range(B):
                    for ck in range(JT):
                        col = ck * B + b
                        for ch in range(CH):
                            nc.tensor.matmul(
                                xs2_ps[:, col : col + 1],
                                x_bf[:, b * CH + ch, ck * 128 : (ck + 1) * 128],
                                ones_red[:],
                                start=(ch == 0),
                                stop=(ch == CH - 1),
                            )
                nc.vector.tensor_copy(xspT_sb[:], xs2_ps[:])

                # ---------------- AllReduce of [B, C] partial Xsum ----------
                ar_in = dram.tile([128, JT * B], f32, name=f"ar_in{_rep}")
                ar_out = dram.tile(
                    [128, JT * B], f32, addr_space="Shared", name=f"ar_out{_rep}"
                )
                bounce = nc.scalar.dma_start(ar_in[:], xspT_sb[:])
                if use_collective:
                    nc.gpsimd.collective_compute(
                        "AllReduce",
                        add,
                        replica_groups=[list(range(NCORES))],
                        ins=[ar_in.opt()],
                        outs=[ar_out.opt()],
                    )
                else:
                    nc.scalar.dma_start(ar_out[:], ar_in[:])
                nc.scalar.dma_start(xsumTf[:], ar_out[:])
                nc.scalar.copy(
                    xsumT_sb[:].rearrange("p ck b -> p (ck b)"), xsumTf[:]
                )

                # ---------------- Wk_h^T via TensorE transpose --------------
                for ck in range(JT):
                    for jt in range(JH):
                        wt_ps = psum.tile(
                            [128, 128], bf16, tag="tr", bufs=2, name=f"wt{jt}_{ck}"
                        )
                        nc.tensor.transpose(
                            wt_ps[:],
                            wk_sb[:, jt, ck * 128 : (ck + 1) * 128],
                            ident_bf[:],
                        )
                        nc.vector.tensor_copy(
                            wkT_sb[:, ck, jt * 128 : (jt + 1) * 128], wt_ps[:]
                        )

                # HAM warm-up: keep the PE clock gate at full rate through the
                # transpose-heavy AllReduce window.
                warm_ps = psum.tile([1, 512], f32, tag="small", bufs=2)
                N_WARM = 12
                for i in range(N_WARM):
                    nc.tensor.matmul(
                        warm_ps[:],
                        ones_red[:],
                        wk_sb[:, i % JH, 0:512],
                        start=(i == 0),
                        stop=(i == N_WARM - 1),
                    )

                # ---------------- X^T via TensorE transpose ----------------
                for b in range(B):
                    for ch in range(CH):
                        col = (b * CH + ch) * 128
                        for ck in range(JT):
                            xt_ps2 = psum.tile(
                                [128, 128], bf16, tag="tr", bufs=2,
                                name=f"xtr{b}_{ch}_{ck}",
                            )
                            nc.tensor.transpose(
                                xt_ps2[:],
                                x_bf[:, b * CH + ch, ck * 128 : (ck + 1) * 128],
                                ident_bf[:],
                            )
                            if ck % 2 == 0:
                                nc.scalar.copy(
                                    xT_sb[:, ck, col : col + 128], xt_ps2[:]
                                )
                            else:
                                nc.vector.tensor_copy(
                                    xT_sb[:, ck, col : col + 128], xt_ps2[:]
                                )

                # ------- t_h[j, b] = sum_c Wk_h^T[c, j] Xsum^T[c, b] --------
                # lhsT = Wk_h^T 128-col slices (stationary), rhs = Xsum^T:
                # lands t directly with j on partitions — no drain/transpose
                for js in range(JH):
                    t_ps = psum.tile(
                        [128, 2], f32, tag="tr", bufs=2, name=f"td{js}"
                    )
                    for ck in range(JT):
                        nc.tensor.matmul(
                            t_ps[:],
                            wkT_sb[:, ck, js * 128 : (js + 1) * 128],
                            xsumT_sb[:, ck, :],
                            start=(ck == 0),
                            stop=(ck == JT - 1),
                        )
                    nc.vector.tensor_copy(t_sb[:, js, :], t_ps[:])

                # ------ u_h^T[c', b] = sum_j Wq_h[j, c'] t_h[j, b] ----------
                # lhsT = Wq_h 128-col slices (stationary), rhs = t_h: lands
                # u^T chunks straight into the send buffer — no drain and no
                # transposes between u and the exchange trigger
                tcopies = []
                for ck in range(JT):
                    u_ps = psum.tile(
                        [128, 2], f32, tag="tr", bufs=2, name=f"ud{ck}"
                    )
                    for jt in range(JH):
                        nc.tensor.matmul(
                            u_ps[:],
                            wq_sb[:, jt, ck * 128 : (ck + 1) * 128],
                            t_sb[:, jt, :],
                            start=(jt == 0),
                            stop=(jt == JH - 1),
                        )
                    cp = nc.vector.tensor_copy(
                        uT_send[:, par, ck * B : (ck + 1) * B], u_ps[:]
                    )
                    tcopies.append(cp)

                # fire the pair exchange once the send buffer is complete
                trig = nc.gpsimd.trigger_dma(count=1, queue_num=1)
                for cp in tcopies:
                    tile.add_dep_helper(trig.ins, cp.ins, sync=True,
                                        reason="u^T drained before trigger")

                # my half casts to bf16 immediately; the partner's cast
                # carries the attached remote-sem wait, so the first six
                # final matmuls (mine) overlap the partner's arrival
                cs_mine = nc.vector.tensor_copy(uTs_bf[:], uT_send[:, par, :])
                cs_part = nc.vector.tensor_copy(uTr_bf[:], uT_recv[:, par, :])
                tile.add_dep_helper(cs_part.ins, trig.ins, sync=False,
                                    reason="partner cast after trigger")
                # the attached rsem wait blocks the DVE stream; every DVE op
                # the AllReduce bounce needs must be scheduled before it,
                # else the cores deadlock through the collective
                tile.add_dep_helper(cs_part.ins, bounce.ins, sync=False,
                                    reason="rsem wait after the CC bounce")
                inject_specs.append(
                    ("vector", rsem, 2 * (_rep + 1), cs_part.ins))

                # ------- scores^T = scale * diag(u^T . X^T) -----------------
                sc_ps = psum.tile([B, 512], f32, tag="mid", bufs=1, name="sc")
                for half, ubuf in ((0, uTs_bf), (1, uTr_bf)):
                    for ck in range(JT):
                        nc.tensor.matmul(
                            sc_ps[:],
                            ubuf[:, ck * B : (ck + 1) * B],
                            xT_sb[:, ck, :],
                            start=(half == 0 and ck == 0),
                            stop=(half == 1 and ck == JT - 1),
                        )
                nc.scalar.mul(out_row2[:, 0:NS], sc_ps[:, 0:NS], SCALE)
                nc.sync.dma_start(out_d[0:1, :], out_row2[0:1, 0:NS])
                nc.vector.tensor_scalar_mul(
                    out_row2[:, NS : 2 * NS], sc_ps[:, NS : 2 * NS], SCALE
                )
                nc.sync.dma_start(out_d[1:2, :], out_row2[1:2, NS : 2 * NS])

    # HW-only cross-core waits, invisible to the scheduling sim
    for eng_name, sem, val, target in inject_specs:
        _attach_wait(target, sem, val)
    # end-of-NEFF quiesce: all sends flushed, all arrivals seen
    nc.gpsimd.wait_ge(lsem, 16 * repeats)
    nc.gpsimd.wait_ge(rsem, 2 * repeats)

    nc.compile()
    return nc


def _get_nc():
    global _compiled_nc
    if _compiled_nc is None:
        _compiled_nc = _build_and_compile()
    return _compiled_nc


def make_in_maps(X, W_qkv):
    import ml_dtypes

    X = np.asarray(X, dtype=np.float32).astype(ml_dtypes.bfloat16)
    W = np.asarray(W_qkv, dtype=np.float32).astype(ml_dtypes.bfloat16)
    assert X.shape == (B, N, C) and W.shape == (2 * C, C)
    halves = []
    for h in range(2):
        wq_h = W[h * 384 : (h + 1) * 384, :]
        wk_h = W[C + h * 384 : C + (h + 1) * 384, :]
        halves.append(np.ascontiguousarray(np.concatenate([wq_h, wk_h], axis=0)))
    return [
        {
            "x_in": np.ascontiguousarray(X[:, i * NS : (i + 1) * NS, :]),
            "w_in": halves[i % 2],
        }
        for i in range(NCORES)
    ]


def assemble_out(results):
    return np.concatenate(
        [results[i]["scores_out"] for i in range(NCORES)], axis=1
    ).astype(np.float32)


def kernel(X, W_qkv):
    from concourse import bass_utils

    nc = _get_nc()
    res = bass_utils.run_bass_kernel_spmd(
        nc, make_in_maps(X, W_qkv), core_ids=list(range(NCORES))
    )
    return assemble_out(res.results)
